# revision 6
# baseline (speedup 1.0000x reference)
"""GPNNCell (gnn_message_passing) Trainium2 Bass kernel, v2.

Full-input contract: kernel(**inputs) takes the complete tensors from
setup_inputs() and returns node_features + sum_w weight_edge * merged_message
-> [8, 64, 768].

Distribution: data-parallel over batch B=8, one batch element per NeuronCore,
no collectives.

v2 redesign vs v0 (what makes it fast):
  * Host pre-transposes/casts edges to X^T [768, 4096] bf16 (w-major cols)
    -> no on-device PE transposes, half the DMA bytes.
  * W_msg @ W_mrg is algebraically fused on host into Wc [768,768] (no
    nonlinearity between message and merge Linear) -> one full 768x768x4096
    matmul stage eliminated. The node-feature half of the message becomes
    P2[w] = (node@Wm_top + b_msg)@W_mrg + b_mrg, computed on host, added on
    device via a K=64 selection matmul into the same psum accumulation.
  * LayerNorm+GELU fused into ONE ACT op: gelu(istd*x - mu*istd) using
    per-partition scale/bias operands; istd via DVE Newton rsqrt (bit-hack
    seed + 2 iterations) so the ACT table set never leaves gelu_and_others
    (tanh/identity/gelu co-resident; Sqrt would cost ~2.7us per swap).
  * weight_edge folded into the neighbor-sum matmul lhsT: J = wt * I2/...,
    acc[v,:] += J.T @ gelu  (psum-resident across the whole kernel).
  * all matmuls bf16 (1 cyc/row warm, FWL weight loads).

Per block of 512 edge rows (8 w x 64 v), per core:
  xt [128,6,512] bf16 <- DMA slice of X^T
  gates^T = Wg[i|g|o].T @ xt          (6 chunks x 6 k x 512 c)
  h2 = (tanh(o/2)+1)*tanh(sig(i)*tanh(g))  (ACT tanh x4/half + DVE stt, bf16)
  pw = (Wl/2).T @ h2 ; wt_t = tanh(pw/2 + bl/2)  (DRAM-bounce reshape)
  m[p,f] = sum_k xt_k.T @ Wc_k + S_idx.T @ P2    (2 psum banks per row-tile)
  stats: bn_stats/bn_aggr; istd = newton_rsqrt(var+eps)
  gl = gelu(istd*m - mu*istd)  bf16
  acc += J.T @ gl,  J = (wt_t+1) * I2/2
out = node + acc.
"""
import numpy as np
import ml_dtypes
from contextlib import ExitStack

import concourse.mybir as mybir
import concourse.tile as tile
from concourse import bacc
from concourse.bass_utils import run_bass_kernel_spmd

F32 = mybir.dt.float32
BF16 = mybir.dt.bfloat16
U32 = mybir.dt.uint32
I32 = mybir.dt.int32
AF = mybir.ActivationFunctionType
OP = mybir.AluOpType

B = 8           # batch == number of cores
N = 64          # nodes
D = 768         # feature dim
H = 256         # lstm hidden
ROWS = N * N    # 4096 edge rows per core
BLK = 512       # rows per block (8 w x 64 v)
NBLK = ROWS // BLK
TPB = BLK // 128  # row-tiles per block
KD = D // 128
LN_EPS = 1e-12
BFDT = ml_dtypes.bfloat16


def build(apply_lng=False, apply_lnb=False, reps=1, v=None):
    v = {**dict(xt_bufs=3, psg_bufs=3, psm_bufs=2, ms_bufs=10, gl_bufs=4,
                tmp_bufs=5, newton=2), **(v or {})}
    nc = bacc.Bacc(None)

    xte = nc.dram_tensor("xte", (D, ROWS), BF16, kind="ExternalInput")
    p2d = nc.dram_tensor("p2", (N, D), BF16, kind="ExternalInput")
    noded = nc.dram_tensor("node", (N, D), F32, kind="ExternalInput")
    wcd = nc.dram_tensor("wc", (D, D), BF16, kind="ExternalInput")
    wgpd = nc.dram_tensor("wgp", (D, D), BF16, kind="ExternalInput")
    bg6d = nc.dram_tensor("bg6", (128, 6), F32, kind="ExternalInput")
    wl2d = nc.dram_tensor("wl2", (H, 1), BF16, kind="ExternalInput")
    bl2d = nc.dram_tensor("bl2", (1,), F32, kind="ExternalInput")
    lngd = nc.dram_tensor("ln_g", (D,), F32, kind="ExternalInput")
    lnbd = nc.dram_tensor("ln_b", (D,), F32, kind="ExternalInput")
    outd = nc.dram_tensor("out", (N, D), F32, kind="ExternalOutput")

    # I2/2 stacked identity [128, 64]: row p -> col p%64, scaled 0.5 so that
    # J = (wt_t+1)*I2/2 = sigmoid(pw+bl)*I2.
    i2h_np = np.tile(np.eye(N, dtype=np.float32), (2, 1)) * 0.5
    i2h_dram = nc.inline_tensor(i2h_np.astype(np.float32), name="i2h")
    # selection stack: S[j, idx*128 + p] = 1 iff j == blk*8 + 2*t + p//64,
    # idx = blk*TPB + t.  lhsT of the P2-add matmul.
    s_np = np.zeros((N, NBLK * TPB, 128), np.float32)
    for idx in range(NBLK * TPB):
        blk, t = divmod(idx, TPB)
        w0 = blk * 8 + 2 * t
        s_np[w0, idx, 0:64] = 1.0
        s_np[w0 + 1, idx, 64:128] = 1.0
    sbig_dram = nc.inline_tensor(s_np.reshape(N, -1).astype(BFDT), name="sbig")

    with tile.TileContext(nc) as tc, ExitStack() as ctx:
        W = ctx.enter_context(tc.tile_pool(name="W", bufs=1))          # persistent
        xtp = ctx.enter_context(tc.tile_pool(name="xt", bufs=v["xt_bufs"]))
        hp = ctx.enter_context(tc.tile_pool(name="h", bufs=2))
        tmp = ctx.enter_context(tc.tile_pool(name="tmp", bufs=v["tmp_bufs"]))
        lnp = ctx.enter_context(tc.tile_pool(name="ln", bufs=v["ms_bufs"]))
        glp = ctx.enter_context(tc.tile_pool(name="gl", bufs=v["gl_bufs"]))
        sml = ctx.enter_context(tc.tile_pool(name="sml", bufs=6))
        drp = ctx.enter_context(tc.tile_pool(name="dr", bufs=2, space="DRAM"))

        psg = ctx.enter_context(tc.tile_pool(name="psg", bufs=v["psg_bufs"], space="PSUM"))
        psm = ctx.enter_context(tc.tile_pool(name="psm", bufs=v["psm_bufs"], space="PSUM"))
        psf = ctx.enter_context(tc.tile_pool(name="psf", bufs=1, space="PSUM"))

        # ---------------- persistent weights / constants ----------------
        # per-k tiles so block 0's first matmuls wait on per-k DMA granularity
        wg_sbs = []
        wc_sbs = []
        for k in range(KD):
            wgk = W.tile([128, D], BF16, tag=f"wg{k}", name=f"wg{k}")
            nc.sync.dma_start(wgk[:], wgpd[k * 128:(k + 1) * 128, :])
            wg_sbs.append(wgk)
        for k in range(KD):
            wck = W.tile([128, D], BF16, tag=f"wc{k}", name=f"wc{k}")
            nc.gpsimd.dma_start(wck[:], wcd[k * 128:(k + 1) * 128, :])
            wc_sbs.append(wck)

        sb_sb = W.tile([N, NBLK * TPB, 128], BF16, tag="sbig")
        nc.gpsimd.dma_start(sb_sb[:], sbig_dram[:].rearrange("j (i r) -> j i r", r=128))
        p2_sb = W.tile([N, D], BF16, tag="p2")
        nc.gpsimd.dma_start(p2_sb[:], p2d[:])

        wl_sb = W.tile([128, 2, 1], BF16, tag="wl")
        nc.sync.dma_start(wl_sb[:], wl2d[:].rearrange("(k p) a -> p k a", p=128))
        bg_sb = W.tile([128, 6], F32, tag="bg")
        nc.sync.dma_start(bg_sb[:], bg6d[:])
        bl2_sb = W.tile([128, 1], F32, tag="bl2")
        nc.sync.dma_start(bl2_sb[:], bl2d[:].partition_broadcast(128))
        i2h_sb = W.tile([128, N], F32, tag="i2h")
        nc.sync.dma_start(i2h_sb[:], i2h_dram[:])
        node_sb = W.tile([N, D], F32, tag="node")
        nc.sync.dma_start(node_sb[:], noded[:])
        if apply_lng:
            gfull = W.tile([128, D], F32, tag="gfull")
            nc.sync.dma_start(gfull[:], lngd[:].partition_broadcast(128))
        if apply_lnb:
            bfull = W.tile([128, D], F32, tag="bfull")
            nc.sync.dma_start(bfull[:], lnbd[:].partition_broadcast(128))

        # final accumulator, one bank: partitions 0:64 = lo half, 64:128 = hi
        acc = psf.tile([128, 384], F32, tag="acc")
        acc_lo = acc[0:N, :]
        acc_hi = acc[N:128, :]

        out_sb = W.tile([N, D], F32, tag="out")

        xview = xte[:].rearrange("(k p) c -> p k c", p=128)

        # ---------------- main loop (body repeated `reps` times) ----------
        def body():
            # software-pipelined: block b's LN+gelu+neighbor-sum ("phase B") is
            # emitted interleaved into block b+1's gates matmuls so the
            # in-order PE queue never waits on the stats->rsqrt->gelu chain.
            def phase_b(st, t):
                (mss, mvall, yb, wt_t, blk) = st
                ms = mss[t]
                gl = glp.tile([128, 2, 384], BF16, tag="gl")
                if not (apply_lng or apply_lnb):
                    nb = sml.tile([128, 1], F32, tag="nb")
                    nc.vector.scalar_tensor_tensor(nb[:], mvall[:, t, 0:1], -1.0,
                                                   yb[:, t:t + 1], OP.mult, OP.mult)
                    nc.scalar.activation(gl[:], ms[:], AF.Gelu,
                                         scale=yb[:, t:t + 1], bias=nb[:])
                else:
                    y = lnp.tile([128, 2, 384], F32, tag="y")
                    nc.vector.tensor_scalar(y[:], ms[:], mvall[:, t, 0:1],
                                            yb[:, t:t + 1], OP.subtract, OP.mult)
                    gview = lambda g: g[:].rearrange("p (h f) -> p h f", h=2)
                    if apply_lng:
                        z = lnp.tile([128, 2, 384], F32, tag="y")
                        nc.vector.tensor_tensor(z[:], y[:], gview(gfull), OP.mult)
                        y = z
                    if apply_lnb:
                        z = lnp.tile([128, 2, 384], F32, tag="y")
                        nc.vector.tensor_tensor(z[:], y[:], gview(bfull), OP.add)
                        y = z
                    nc.scalar.activation(gl[:], y[:], AF.Gelu)

                J = sml.tile([128, N], BF16, tag="J")
                nc.vector.scalar_tensor_tensor(
                    J[:], wt_t[:, t:t + 1].broadcast_to((128, N)), 1.0,
                    i2h_sb[:], OP.add, OP.mult)
                first = blk == 0 and t == 0
                last = blk == NBLK - 1 and t == TPB - 1
                nc.tensor.matmul(acc_lo, J[:], gl[:, 0, :],
                                 start=first, stop=last, skip_group_check=True)
                nc.tensor.matmul(acc_hi, J[:], gl[:, 1, :],
                                 start=first, stop=last, skip_group_check=True)

            prev = None
            for blk in range(NBLK):
                xt = xtp.tile([128, KD, BLK], BF16, tag="xt")
                nc.sync.dma_start(xt[:, 0:3, :],
                                  xview[:, 0:3, blk * BLK:(blk + 1) * BLK])
                nc.gpsimd.dma_start(xt[:, 3:6, :],
                                    xview[:, 3:6, blk * BLK:(blk + 1) * BLK])

                # ---- gates (all-tanh rewrite; sig(x)=0.5*tanh(x/2)+0.5) ----
                # chunk order in wgp/bg6: i0 i1 g0 g1 o0 o1 (i,o biases halved)
                def gate_mm(cj):
                    pg = psg.tile([128, BLK], F32, tag="s1")
                    for k in range(KD):
                        nc.tensor.matmul(pg[:], wg_sbs[k][:, cj * 128:(cj + 1) * 128],
                                         xt[:, k, :], start=(k == 0), stop=(k == KD - 1))
                    return pg

                h_sb = hp.tile([128, 2, BLK], BF16, tag="h")
                for half in range(2):
                    pg_i = gate_mm(half)
                    tan_i = tmp.tile([128, BLK], BF16, tag="tmp")
                    nc.scalar.activation(tan_i[:], pg_i[:], AF.Tanh, scale=0.5,
                                         bias=bg_sb[:, half:half + 1])
                    pg_g = gate_mm(2 + half)
                    tan_g = tmp.tile([128, BLK], BF16, tag="tmp")
                    nc.scalar.activation(tan_g[:], pg_g[:], AF.Tanh,
                                         bias=bg_sb[:, 2 + half:3 + half])
                    c_t = tmp.tile([128, BLK], BF16, tag="tmp")
                    nc.vector.scalar_tensor_tensor(c_t[:], tan_i[:], 1.0, tan_g[:],
                                                   OP.add, OP.mult)
                    tan_c = tmp.tile([128, BLK], BF16, tag="tmp")
                    nc.scalar.activation(tan_c[:], c_t[:], AF.Tanh, scale=0.5)
                    pg_o = gate_mm(4 + half)
                    tan_o = tmp.tile([128, BLK], BF16, tag="tmp")
                    nc.scalar.activation(tan_o[:], pg_o[:], AF.Tanh, scale=0.5,
                                         bias=bg_sb[:, 4 + half:5 + half])
                    nc.vector.scalar_tensor_tensor(h_sb[:, half, :], tan_o[:], 1.0,
                                                   tan_c[:], OP.add, OP.mult)
                    # interleave prev block's gelu+acc with this block's gates
                    if prev is not None:
                        phase_b(prev, 2 * half)
                        phase_b(prev, 2 * half + 1)

                # ---- merge: m = X @ Wc + P2[w]; stats per row-tile ----
                mss = []
                mvall = sml.tile([128, TPB, 2], F32, tag="mv")
                for t in range(TPB):
                    idx = blk * TPB + t
                    pm = psm.tile([128, 2, BLK], F32, tag="pm")
                    for hf in range(2):
                        o = pm[:, hf, 0:384]
                        nc.tensor.matmul(o, sb_sb[:, idx, :],
                                         p2_sb[:, hf * 384:(hf + 1) * 384],
                                         start=True, stop=False)
                        for k in range(KD):
                            nc.tensor.matmul(o, xt[:, k, t * 128:(t + 1) * 128],
                                             wc_sbs[k][:, hf * 384:(hf + 1) * 384],
                                             start=False, stop=(k == KD - 1))
                    ms = lnp.tile([128, 2, 384], F32, tag="ms", name=f"ms_{blk}_{t}")
                    nc.scalar.activation(ms[:], pm[:, :, 0:384], AF.Identity)
                    stats = sml.tile([128, 2, 6], F32, tag="st")
                    nc.vector.bn_stats(stats[:, 0, :], ms[:, 0, :])
                    nc.vector.bn_stats(stats[:, 1, :], ms[:, 1, :])
                    nc.vector.bn_aggr(mvall[:, t, :], stats[:])
                    mss.append(ms)

                # ---- edge weight: pw = (Wl/2).T @ h2 -> [1, 512] ----
                # (emitted after merge so the PE never waits on the tanh chain)
                pgw = psg.tile([128, BLK], F32, tag="s1")
                pw = pgw[0:1, :]
                for k in range(2):
                    nc.tensor.matmul(pw, wl_sb[:, k, :], h_sb[:, k, :],
                                     start=(k == 0), stop=(k == 1))
                wrow = sml.tile([1, BLK], F32, tag="wrow")
                nc.vector.tensor_copy(wrow[:], pw)
                wdr = drp.tile([1, BLK], F32, tag="wdr")
                nc.gpsimd.dma_start(wdr[:], wrow[:])
                wtp = sml.tile([128, TPB], F32, tag="wtp")
                nc.gpsimd.dma_start(wtp[:],
                                    wdr[0:1, :].rearrange("a (t p) -> (a p) t", p=128))
                wt_t = sml.tile([128, TPB], F32, tag="wtt")
                nc.scalar.activation(wt_t[:], wtp[:], AF.Tanh, scale=0.5,
                                     bias=bl2_sb[:])

                # ---- istd = rsqrt(var + eps): bit-hack seed + newton ----
                veps = sml.tile([128, TPB], F32, tag="veps")
                nc.vector.tensor_scalar(veps[:], mvall[:, :, 1], LN_EPS, None, OP.add)
                yb = sml.tile([128, TPB], F32, tag="yb")
                # y0 = bits(0x5f3759df - (bits(v) >> 1)) = magic+1 + ~(v>>1)
                nc.vector.tensor_scalar(yb[:].bitcast(U32), veps[:].bitcast(U32),
                                        1, 0xFFFFFFFF,
                                        OP.logical_shift_right, OP.bitwise_xor)
                # int32 add: uint32 ALU add saturates on device, int32 is
                # in-range here (~(v>>1) is negative, result positive)
                nc.vector.tensor_scalar(yb[:].bitcast(I32), yb[:].bitcast(I32),
                                        0x5f3759e0, None, OP.add)
                for _ in range(v["newton"]):
                    a = sml.tile([128, TPB], F32, tag="nta")
                    nc.vector.tensor_tensor(a[:], veps[:], yb[:], OP.mult)
                    nc.vector.tensor_tensor(a[:], a[:], yb[:], OP.mult)
                    nc.vector.tensor_scalar(a[:], a[:], -0.5, 1.5, OP.mult, OP.add)
                    nc.vector.tensor_tensor(yb[:], yb[:], a[:], OP.mult)

                prev = (mss, mvall, yb, wt_t, blk)

            for t in range(TPB):
                phase_b(prev, t)

            # ---- residual + store ----
            nc.vector.scalar_tensor_tensor(out_sb[:, 0:384], acc_lo, 0.0,
                                           node_sb[:, 0:384], OP.add, OP.add)
            nc.vector.scalar_tensor_tensor(out_sb[:, 384:768], acc_hi, 0.0,
                                           node_sb[:, 384:768], OP.add, OP.add)
            nc.sync.dma_start(outd[:], out_sb[:])

        if reps == 1:
            body()
        else:
            with tc.For_i(0, reps, 1):
                body()

    nc.finalize()
    return nc


_CACHE = {}
VOPT = None


def _get_nc(flags, reps=1):
    key = (flags, reps, repr(VOPT))
    if key not in _CACHE:
        _CACHE[key] = build(apply_lng=flags[0], apply_lnb=flags[1],
                            reps=reps, v=VOPT)
    return _CACHE[key]


def _flags(inputs):
    return (not bool(np.allclose(inputs["ln_g"], 1.0)),
            bool(np.any(inputs["ln_b"])))


def _in_maps(inputs):
    e = np.asarray(inputs["edge_features"], np.float32)       # [8,64,64,768]
    nf = np.ascontiguousarray(inputs["node_features"], np.float32)
    Wg = np.asarray(inputs["W_gates"], np.float32)            # [768,1024]
    bgv = np.asarray(inputs["b_gates"], np.float32)
    Wl = np.asarray(inputs["W_lout"], np.float32)             # [256,1]
    blv = np.asarray(inputs["b_lout"], np.float32)
    Wm = np.asarray(inputs["W_msg"], np.float32)              # [1536,768]
    bmv = np.asarray(inputs["b_msg"], np.float32)
    Wr = np.asarray(inputs["W_mrg"], np.float32)              # [768,768]
    brv = np.asarray(inputs["b_mrg"], np.float32)

    # X^T per core: [768, 4096] bf16, column = w*64 + v
    xT = np.ascontiguousarray(e.transpose(0, 3, 2, 1)).reshape(B, D, ROWS).astype(BFDT)
    # P2[w] = (node_w @ Wm_top + b_msg) @ W_mrg + b_mrg  (per core, host)
    p2 = (((nf @ Wm[:D] + bmv) @ Wr) + brv).astype(BFDT)      # [8,64,768]
    # fused message-bottom x merge weight
    wc = (Wm[D:].astype(np.float64) @ Wr.astype(np.float64)).astype(np.float32).astype(BFDT)
    # gates packed [i|g|o] (f-gate dropped: c0 = 0)
    wgp = np.ascontiguousarray(
        np.concatenate([Wg[:, 0:256], Wg[:, 512:1024]], axis=1)).astype(BFDT)
    bg6 = np.stack([bgv[0:128] * .5, bgv[128:256] * .5, bgv[512:640],
                    bgv[640:768], bgv[768:896] * .5, bgv[896:1024] * .5],
                   axis=1).astype(np.float32)                 # [128, 6]
    wl2 = (Wl * 0.5).astype(BFDT)
    bl2 = (blv * 0.5).astype(np.float32)
    lg = np.ascontiguousarray(inputs["ln_g"], np.float32)
    lb = np.ascontiguousarray(inputs["ln_b"], np.float32)

    shared = dict(wc=wc, wgp=wgp, bg6=bg6, wl2=wl2, bl2=bl2, ln_g=lg, ln_b=lb)
    return [dict(xte=xT[b], p2=p2[b], node=nf[b], **shared) for b in range(B)]


def kernel(**inputs):
    nc = _get_nc(_flags(inputs))
    res = run_bass_kernel_spmd(nc, _in_maps(inputs), list(range(B)))
    return np.stack([res.results[b]["out"] for b in range(B)]).astype(np.float32)


def run_timed(inputs, reps):
    """Run the reps-looped variant once; returns (output, wall_seconds)."""
    import time
    nc = _get_nc(_flags(inputs), reps=reps)
    maps = _in_maps(inputs)
    t0 = time.time()
    res = run_bass_kernel_spmd(nc, maps, list(range(B)))
    dt = time.time() - t0
    out = np.stack([res.results[b]["out"] for b in range(B)]).astype(np.float32)
    return out, dt


# revision 17
# speedup vs baseline: 1.2936x; 1.2936x over previous
"""GPNNCell (gnn_message_passing) Trainium2 Bass kernel, v2.

Full-input contract: kernel(**inputs) takes the complete tensors from
setup_inputs() and returns node_features + sum_w weight_edge * merged_message
-> [8, 64, 768].

Distribution: data-parallel over batch B=8, one batch element per NeuronCore,
no collectives.

v2 redesign vs v0 (what makes it fast):
  * Host pre-transposes/casts edges to X^T [768, 4096] bf16 (w-major cols)
    -> no on-device PE transposes, half the DMA bytes.
  * W_msg @ W_mrg is algebraically fused on host into Wc [768,768] (no
    nonlinearity between message and merge Linear) -> one full 768x768x4096
    matmul stage eliminated. The node-feature half of the message becomes
    P2[w] = (node@Wm_top + b_msg)@W_mrg + b_mrg, computed on host, added on
    device via a K=64 selection matmul into the same psum accumulation.
  * LayerNorm+GELU fused into ONE ACT op: gelu(istd*x - mu*istd) using
    per-partition scale/bias operands; istd via DVE Newton rsqrt (bit-hack
    seed + 2 iterations) so the ACT table set never leaves gelu_and_others
    (tanh/identity/gelu co-resident; Sqrt would cost ~2.7us per swap).
  * weight_edge folded into the neighbor-sum matmul lhsT: J = wt * I2/...,
    acc[v,:] += J.T @ gelu  (psum-resident across the whole kernel).
  * all matmuls bf16 (1 cyc/row warm, FWL weight loads).

Per block of 512 edge rows (8 w x 64 v), per core:
  xt [128,6,512] bf16 <- DMA slice of X^T
  gates^T = Wg[i|g|o].T @ xt          (6 chunks x 6 k x 512 c)
  h2 = (tanh(o/2)+1)*tanh(sig(i)*tanh(g))  (ACT tanh x4/half + DVE stt, bf16)
  pw = (Wl/2).T @ h2 ; wt_t = tanh(pw/2 + bl/2)  (DRAM-bounce reshape)
  m[p,f] = sum_k xt_k.T @ Wc_k + S_idx.T @ P2    (2 psum banks per row-tile)
  stats: bn_stats/bn_aggr; istd = newton_rsqrt(var+eps)
  gl = gelu(istd*m - mu*istd)  bf16
  acc += J.T @ gl,  J = (wt_t+1) * I2/2
out = node + acc.
"""
import numpy as np
import ml_dtypes
from contextlib import ExitStack

import concourse.mybir as mybir
import concourse.tile as tile
from concourse import bacc
from concourse.bass_utils import run_bass_kernel_spmd

F32 = mybir.dt.float32
BF16 = mybir.dt.bfloat16
FP8 = mybir.dt.float8e4
U32 = mybir.dt.uint32
I32 = mybir.dt.int32
AF = mybir.ActivationFunctionType
OP = mybir.AluOpType
DR = mybir.MatmulPerfMode.DoubleRow

B = 8           # batch == number of cores
N = 64          # nodes
D = 768         # feature dim
H = 256         # lstm hidden
ROWS = N * N    # 4096 edge rows per core
BLK = 512       # rows per block (8 w x 64 v)
NBLK = ROWS // BLK
TPB = BLK // 128  # row-tiles per block
KD = D // 128
LN_EPS = 1e-12
BFDT = ml_dtypes.bfloat16


def build(apply_lng=False, apply_lnb=False, reps=1, v=None):
    v = {**dict(xt_bufs=3, psg_bufs=3, psm_bufs=2, ms_bufs=10, gl_bufs=4,
                tmp_bufs=5, newton=2, fp8=True), **(v or {})}
    nc = bacc.Bacc(None)

    xte = nc.dram_tensor("xte", (D, ROWS), BF16, kind="ExternalInput")
    # fp8 copies for the DoubleRow gates matmuls: rows ordered
    # (k2, i, p) -> feature k2*256 + i*128 + p, matching wg8 packing
    x8e = nc.dram_tensor("x8e", (D, ROWS), FP8, kind="ExternalInput")
    wg8d = nc.dram_tensor("wg8", (128, KD // 2, 2, D), FP8, kind="ExternalInput")
    p2d = nc.dram_tensor("p2", (N, D), BF16, kind="ExternalInput")
    noded = nc.dram_tensor("node", (N, D), F32, kind="ExternalInput")
    wcd = nc.dram_tensor("wc", (D, D), BF16, kind="ExternalInput")
    wgpd = nc.dram_tensor("wgp", (D, D), BF16, kind="ExternalInput")
    bg6d = nc.dram_tensor("bg6", (128, 6), F32, kind="ExternalInput")
    wl2d = nc.dram_tensor("wl2", (H, 1), BF16, kind="ExternalInput")
    bl2d = nc.dram_tensor("bl2", (1,), F32, kind="ExternalInput")
    lngd = nc.dram_tensor("ln_g", (D,), F32, kind="ExternalInput")
    lnbd = nc.dram_tensor("ln_b", (D,), F32, kind="ExternalInput")
    outd = nc.dram_tensor("out", (N, D), F32, kind="ExternalOutput")

    # I2/2 stacked identity [128, 64]: row p -> col p%64, scaled 0.5 so that
    # J = (wt_t+1)*I2/2 = sigmoid(pw+bl)*I2.
    i2h_np = np.tile(np.eye(N, dtype=np.float32), (2, 1)) * 0.5
    i2h_dram = nc.inline_tensor(i2h_np.astype(np.float32), name="i2h")
    # selection stack: S[j, idx*128 + p] = 1 iff j == blk*8 + 2*t + p//64,
    # idx = blk*TPB + t.  lhsT of the P2-add matmul.
    s_np = np.zeros((N, NBLK * TPB, 128), np.float32)
    for idx in range(NBLK * TPB):
        blk, t = divmod(idx, TPB)
        w0 = blk * 8 + 2 * t
        s_np[w0, idx, 0:64] = 1.0
        s_np[w0 + 1, idx, 64:128] = 1.0
    sbig_dram = nc.inline_tensor(s_np.reshape(N, -1).astype(BFDT), name="sbig")

    with tile.TileContext(nc) as tc, ExitStack() as ctx:
        W = ctx.enter_context(tc.tile_pool(name="W", bufs=1))          # persistent
        xtp = ctx.enter_context(tc.tile_pool(name="xt", bufs=v["xt_bufs"]))
        hp = ctx.enter_context(tc.tile_pool(name="h", bufs=2))
        tmp = ctx.enter_context(tc.tile_pool(name="tmp", bufs=v["tmp_bufs"]))
        lnp = ctx.enter_context(tc.tile_pool(name="ln", bufs=v["ms_bufs"]))
        glp = ctx.enter_context(tc.tile_pool(name="gl", bufs=v["gl_bufs"]))
        sml = ctx.enter_context(tc.tile_pool(name="sml", bufs=6))
        drp = ctx.enter_context(tc.tile_pool(name="dr", bufs=2, space="DRAM"))

        psg = ctx.enter_context(tc.tile_pool(name="psg", bufs=v["psg_bufs"], space="PSUM"))
        psm = ctx.enter_context(tc.tile_pool(name="psm", bufs=v["psm_bufs"], space="PSUM"))
        psf = ctx.enter_context(tc.tile_pool(name="psf", bufs=1, space="PSUM"))

        # ---------------- persistent weights / constants ----------------
        # per-k tiles so block 0's first matmuls wait on per-k DMA granularity
        wg_sbs = []
        wc_sbs = []
        if v["fp8"]:
            wg8_sb = W.tile([128, KD // 2, 2, D], FP8, tag="wg8")
            nc.sync.dma_start(wg8_sb[:], wg8d[:])
        else:
            for k in range(KD):
                wgk = W.tile([128, D], BF16, tag=f"wg{k}", name=f"wg{k}")
                nc.sync.dma_start(wgk[:], wgpd[k * 128:(k + 1) * 128, :])
                wg_sbs.append(wgk)
        for k in range(KD):
            wck = W.tile([128, D], BF16, tag=f"wc{k}", name=f"wc{k}")
            nc.gpsimd.dma_start(wck[:], wcd[k * 128:(k + 1) * 128, :])
            wc_sbs.append(wck)

        sb_sb = W.tile([N, NBLK * TPB, 128], BF16, tag="sbig")
        nc.gpsimd.dma_start(sb_sb[:], sbig_dram[:].rearrange("j (i r) -> j i r", r=128))
        p2_sb = W.tile([N, D], BF16, tag="p2")
        nc.gpsimd.dma_start(p2_sb[:], p2d[:])

        wl_sb = W.tile([128, 2, 1], BF16, tag="wl")
        nc.sync.dma_start(wl_sb[:], wl2d[:].rearrange("(k p) a -> p k a", p=128))
        bg_sb = W.tile([128, 6], F32, tag="bg")
        nc.sync.dma_start(bg_sb[:], bg6d[:])
        bl2_sb = W.tile([128, 1], F32, tag="bl2")
        nc.sync.dma_start(bl2_sb[:], bl2d[:].partition_broadcast(128))
        i2h_sb = W.tile([128, N], F32, tag="i2h")
        nc.sync.dma_start(i2h_sb[:], i2h_dram[:])
        node_sb = W.tile([N, D], F32, tag="node")
        nc.sync.dma_start(node_sb[:], noded[:])
        if apply_lng:
            gfull = W.tile([128, D], F32, tag="gfull")
            nc.sync.dma_start(gfull[:], lngd[:].partition_broadcast(128))
        if apply_lnb:
            bfull = W.tile([128, D], F32, tag="bfull")
            nc.sync.dma_start(bfull[:], lnbd[:].partition_broadcast(128))

        # final accumulator, one bank: partitions 0:64 = lo half, 64:128 = hi
        acc = psf.tile([128, 384], F32, tag="acc")
        acc_lo = acc[0:N, :]
        acc_hi = acc[N:128, :]

        out_sb = W.tile([N, D], F32, tag="out")

        xview = xte[:].rearrange("(k p) c -> p k c", p=128)
        if v["fp8"]:
            x8view = x8e[:].rearrange("(k2 i p) c -> p k2 i c", p=128, i=2)

        # ---------------- main loop (body repeated `reps` times) ----------
        def body():
            # software-pipelined: block b's LN+gelu+neighbor-sum ("phase B") is
            # emitted interleaved into block b+1's gates matmuls so the
            # in-order PE queue never waits on the stats->rsqrt->gelu chain.
            def phase_b(st, t):
                (mss, mvall, yb, nba, wt_t, blk) = st
                ms = mss[t]
                gl = glp.tile([128, 2, 384], BF16, tag="gl")
                if not (apply_lng or apply_lnb):
                    nc.scalar.activation(gl[:], ms[:], AF.Gelu,
                                         scale=yb[:, t:t + 1], bias=nba[:, t:t + 1])
                else:
                    y = lnp.tile([128, 2, 384], F32, tag="y")
                    nc.vector.tensor_scalar(y[:], ms[:], mvall[:, t, 0:1],
                                            yb[:, t:t + 1], OP.subtract, OP.mult)
                    gview = lambda g: g[:].rearrange("p (h f) -> p h f", h=2)
                    if apply_lng:
                        z = lnp.tile([128, 2, 384], F32, tag="y")
                        nc.vector.tensor_tensor(z[:], y[:], gview(gfull), OP.mult)
                        y = z
                    if apply_lnb:
                        z = lnp.tile([128, 2, 384], F32, tag="y")
                        nc.vector.tensor_tensor(z[:], y[:], gview(bfull), OP.add)
                        y = z
                    nc.scalar.activation(gl[:], y[:], AF.Gelu)

                J = sml.tile([128, N], BF16, tag="J")
                nc.vector.scalar_tensor_tensor(
                    J[:], wt_t[:, t:t + 1].broadcast_to((128, N)), 1.0,
                    i2h_sb[:], OP.add, OP.mult)
                first = blk == 0 and t == 0
                last = blk == NBLK - 1 and t == TPB - 1
                nc.tensor.matmul(acc_lo, J[:], gl[:, 0, :],
                                 start=first, stop=last, skip_group_check=True)
                nc.tensor.matmul(acc_hi, J[:], gl[:, 1, :],
                                 start=first, stop=last, skip_group_check=True)

            prev = None
            for blk in range(NBLK):
                xt = xtp.tile([128, KD, BLK], BF16, tag="xt")
                nc.sync.dma_start(xt[:, 0:3, :],
                                  xview[:, 0:3, blk * BLK:(blk + 1) * BLK])
                nc.gpsimd.dma_start(xt[:, 3:6, :],
                                    xview[:, 3:6, blk * BLK:(blk + 1) * BLK])
                if v["fp8"]:
                    xt8 = xtp.tile([128, KD // 2, 2, BLK], FP8, tag="xt8")
                    nc.sync.dma_start(xt8[:], x8view[:, :, :, blk * BLK:(blk + 1) * BLK])

                # ---- gates (all-tanh rewrite; sig(x)=0.5*tanh(x/2)+0.5) ----
                # chunk order in wgp/bg6: i0 i1 g0 g1 o0 o1 (i,o biases halved)
                def gate_mm(cj):
                    pg = psg.tile([128, BLK], F32, tag="s1")
                    if v["fp8"]:
                        for k2 in range(KD // 2):
                            nc.tensor.matmul(pg[:],
                                             wg8_sb[:, k2, :, cj * 128:(cj + 1) * 128],
                                             xt8[:, k2, :, :], start=(k2 == 0),
                                             stop=(k2 == KD // 2 - 1), perf_mode=DR)
                    else:
                        for k in range(KD):
                            nc.tensor.matmul(pg[:], wg_sbs[k][:, cj * 128:(cj + 1) * 128],
                                             xt[:, k, :], start=(k == 0), stop=(k == KD - 1))
                    return pg

                h_sb = hp.tile([128, 2, BLK], BF16, tag="h")
                for half in range(2):
                    pg_i = gate_mm(half)
                    tan_i = tmp.tile([128, BLK], BF16, tag="tmp")
                    nc.scalar.activation(tan_i[:], pg_i[:], AF.Tanh, scale=0.5,
                                         bias=bg_sb[:, half:half + 1])
                    pg_g = gate_mm(2 + half)
                    tan_g = tmp.tile([128, BLK], BF16, tag="tmp")
                    nc.scalar.activation(tan_g[:], pg_g[:], AF.Tanh,
                                         bias=bg_sb[:, 2 + half:3 + half])
                    c_t = tmp.tile([128, BLK], BF16, tag="tmp")
                    nc.vector.scalar_tensor_tensor(c_t[:], tan_i[:], 1.0, tan_g[:],
                                                   OP.add, OP.mult)
                    tan_c = tmp.tile([128, BLK], BF16, tag="tmp")
                    nc.scalar.activation(tan_c[:], c_t[:], AF.Tanh, scale=0.5)
                    pg_o = gate_mm(4 + half)
                    tan_o = tmp.tile([128, BLK], BF16, tag="tmp")
                    nc.scalar.activation(tan_o[:], pg_o[:], AF.Tanh, scale=0.5,
                                         bias=bg_sb[:, 4 + half:5 + half])
                    nc.vector.scalar_tensor_tensor(h_sb[:, half, :], tan_o[:], 1.0,
                                                   tan_c[:], OP.add, OP.mult)
                    # interleave prev block's gelu+acc with this block's gates
                    if prev is not None:
                        phase_b(prev, 2 * half)
                        phase_b(prev, 2 * half + 1)

                # ---- merge: m = X @ Wc + P2[w]; stats per row-tile ----
                mss = []
                mvall = sml.tile([128, TPB, 2], F32, tag="mv")
                for t in range(TPB):
                    idx = blk * TPB + t
                    pm = psm.tile([128, 2, BLK], F32, tag="pm")
                    for hf in range(2):
                        o = pm[:, hf, 0:384]
                        nc.tensor.matmul(o, sb_sb[:, idx, :],
                                         p2_sb[:, hf * 384:(hf + 1) * 384],
                                         start=True, stop=False)
                        for k in range(KD):
                            nc.tensor.matmul(o, xt[:, k, t * 128:(t + 1) * 128],
                                             wc_sbs[k][:, hf * 384:(hf + 1) * 384],
                                             start=False, stop=(k == KD - 1))
                    ms = lnp.tile([128, 2, 384], F32, tag="ms", name=f"ms_{blk}_{t}")
                    nc.scalar.activation(ms[:], pm[:, :, 0:384], AF.Identity)
                    stats = sml.tile([128, 2, 6], F32, tag="st")
                    nc.vector.bn_stats(stats[:, 0, :], ms[:, 0, :])
                    nc.vector.bn_stats(stats[:, 1, :], ms[:, 1, :])
                    nc.vector.bn_aggr(mvall[:, t, :], stats[:])
                    mss.append(ms)

                # ---- edge weight: pw = (Wl/2).T @ h2 -> [1, 512] ----
                # (emitted after merge so the PE never waits on the tanh chain)
                pgw = psg.tile([128, BLK], F32, tag="s1")
                pw = pgw[0:1, :]
                for k in range(2):
                    nc.tensor.matmul(pw, wl_sb[:, k, :], h_sb[:, k, :],
                                     start=(k == 0), stop=(k == 1))
                wrow = sml.tile([1, BLK], F32, tag="wrow")
                nc.vector.tensor_copy(wrow[:], pw)
                wdr = drp.tile([1, BLK], F32, tag="wdr")
                nc.gpsimd.dma_start(wdr[:], wrow[:])
                wtp = sml.tile([128, TPB], F32, tag="wtp")
                nc.gpsimd.dma_start(wtp[:],
                                    wdr[0:1, :].rearrange("a (t p) -> (a p) t", p=128))
                wt_t = sml.tile([128, TPB], F32, tag="wtt")
                nc.scalar.activation(wt_t[:], wtp[:], AF.Tanh, scale=0.5,
                                     bias=bl2_sb[:])

                # ---- istd = rsqrt(var + eps): bit-hack seed + newton ----
                veps = sml.tile([128, TPB], F32, tag="veps")
                nc.vector.tensor_scalar(veps[:], mvall[:, :, 1], LN_EPS, None, OP.add)
                yb = sml.tile([128, TPB], F32, tag="yb")
                # y0 = bits(0x5f3759df - (bits(v) >> 1)) = magic+1 + ~(v>>1)
                nc.vector.tensor_scalar(yb[:].bitcast(U32), veps[:].bitcast(U32),
                                        1, 0xFFFFFFFF,
                                        OP.logical_shift_right, OP.bitwise_xor)
                # int32 add: uint32 ALU add saturates on device, int32 is
                # in-range here (~(v>>1) is negative, result positive)
                nc.vector.tensor_scalar(yb[:].bitcast(I32), yb[:].bitcast(I32),
                                        0x5f3759e0, None, OP.add)
                for _ in range(v["newton"]):
                    a = sml.tile([128, TPB], F32, tag="nta")
                    nc.vector.tensor_tensor(a[:], veps[:], yb[:], OP.mult)
                    nc.vector.tensor_tensor(a[:], a[:], yb[:], OP.mult)
                    nc.vector.tensor_scalar(a[:], a[:], -0.5, 1.5, OP.mult, OP.add)
                    nc.vector.tensor_tensor(yb[:], yb[:], a[:], OP.mult)

                prev = (mss, mvall, yb, wt_t, blk)

            for t in range(TPB):
                phase_b(prev, t)

            # ---- residual + store ----
            nc.vector.scalar_tensor_tensor(out_sb[:, 0:384], acc_lo, 0.0,
                                           node_sb[:, 0:384], OP.add, OP.add)
            nc.vector.scalar_tensor_tensor(out_sb[:, 384:768], acc_hi, 0.0,
                                           node_sb[:, 384:768], OP.add, OP.add)
            nc.sync.dma_start(outd[:], out_sb[:])

        if reps == 1:
            body()
        else:
            with tc.For_i(0, reps, 1):
                body()

    nc.finalize()
    return nc


_CACHE = {}
VOPT = None


def _get_nc(flags, reps=1):
    key = (flags, reps, repr(VOPT))
    if key not in _CACHE:
        _CACHE[key] = build(apply_lng=flags[0], apply_lnb=flags[1],
                            reps=reps, v=VOPT)
    return _CACHE[key]


def _flags(inputs):
    return (not bool(np.allclose(inputs["ln_g"], 1.0)),
            bool(np.any(inputs["ln_b"])))


def _in_maps(inputs):
    e = np.asarray(inputs["edge_features"], np.float32)       # [8,64,64,768]
    nf = np.ascontiguousarray(inputs["node_features"], np.float32)
    Wg = np.asarray(inputs["W_gates"], np.float32)            # [768,1024]
    bgv = np.asarray(inputs["b_gates"], np.float32)
    Wl = np.asarray(inputs["W_lout"], np.float32)             # [256,1]
    blv = np.asarray(inputs["b_lout"], np.float32)
    Wm = np.asarray(inputs["W_msg"], np.float32)              # [1536,768]
    bmv = np.asarray(inputs["b_msg"], np.float32)
    Wr = np.asarray(inputs["W_mrg"], np.float32)              # [768,768]
    brv = np.asarray(inputs["b_mrg"], np.float32)

    # X^T per core: [768, 4096] bf16, column = w*64 + v
    xTf = np.ascontiguousarray(e.transpose(0, 3, 2, 1)).reshape(B, D, ROWS)
    xT = xTf.astype(BFDT)
    x8 = xTf.astype(ml_dtypes.float8_e4m3)
    # P2[w] = (node_w @ Wm_top + b_msg) @ W_mrg + b_mrg  (per core, host)
    p2 = (((nf @ Wm[:D] + bmv) @ Wr) + brv).astype(BFDT)      # [8,64,768]
    # fused message-bottom x merge weight
    wc = (Wm[D:].astype(np.float64) @ Wr.astype(np.float64)).astype(np.float32).astype(BFDT)
    # gates packed [i|g|o] (f-gate dropped: c0 = 0)
    wgpf = np.ascontiguousarray(
        np.concatenate([Wg[:, 0:256], Wg[:, 512:1024]], axis=1))
    wgp = wgpf.astype(BFDT)
    # DoubleRow packing: wg8[p, k2, i, m] = wgp[k2*256 + i*128 + p, m]
    wg8 = np.ascontiguousarray(
        wgpf.reshape(3, 2, 128, D).transpose(2, 0, 1, 3)).astype(ml_dtypes.float8_e4m3)
    bg6 = np.stack([bgv[0:128] * .5, bgv[128:256] * .5, bgv[512:640],
                    bgv[640:768], bgv[768:896] * .5, bgv[896:1024] * .5],
                   axis=1).astype(np.float32)                 # [128, 6]
    wl2 = (Wl * 0.5).astype(BFDT)
    bl2 = (blv * 0.5).astype(np.float32)
    lg = np.ascontiguousarray(inputs["ln_g"], np.float32)
    lb = np.ascontiguousarray(inputs["ln_b"], np.float32)

    shared = dict(wc=wc, wgp=wgp, wg8=wg8, bg6=bg6, wl2=wl2, bl2=bl2,
                  ln_g=lg, ln_b=lb)
    return [dict(xte=xT[b], x8e=x8[b], p2=p2[b], node=nf[b], **shared)
            for b in range(B)]


def kernel(**inputs):
    nc = _get_nc(_flags(inputs))
    res = run_bass_kernel_spmd(nc, _in_maps(inputs), list(range(B)))
    return np.stack([res.results[b]["out"] for b in range(B)]).astype(np.float32)


def run_timed(inputs, reps):
    """Run the reps-looped variant once; returns (output, wall_seconds)."""
    import time
    nc = _get_nc(_flags(inputs), reps=reps)
    maps = _in_maps(inputs)
    t0 = time.time()
    res = run_bass_kernel_spmd(nc, maps, list(range(B)))
    dt = time.time() - t0
    out = np.stack([res.results[b]["out"] for b in range(B)]).astype(np.float32)
    return out, dt


# revision 38
# speedup vs baseline: 2.4831x; 1.9195x over previous
"""GPNNCell (gnn_message_passing) Trainium2 Bass kernel, v2.

Full-input contract: kernel(**inputs) takes the complete tensors from
setup_inputs() and returns node_features + sum_w weight_edge * merged_message
-> [8, 64, 768].

Distribution: data-parallel over batch B=8, one batch element per NeuronCore,
no collectives.

v2 redesign vs v0 (what makes it fast):
  * Host pre-transposes/casts edges to X^T [768, 4096] bf16 (w-major cols)
    -> no on-device PE transposes, half the DMA bytes.
  * W_msg @ W_mrg is algebraically fused on host into Wc [768,768] (no
    nonlinearity between message and merge Linear) -> one full 768x768x4096
    matmul stage eliminated. The node-feature half of the message becomes
    P2[w] = (node@Wm_top + b_msg)@W_mrg + b_mrg, computed on host, added on
    device via a K=64 selection matmul into the same psum accumulation.
  * LayerNorm+GELU fused into ONE ACT op: gelu(istd*x - mu*istd) using
    per-partition scale/bias operands; istd via DVE Newton rsqrt (bit-hack
    seed + 2 iterations) so the ACT table set never leaves gelu_and_others
    (tanh/identity/gelu co-resident; Sqrt would cost ~2.7us per swap).
  * weight_edge folded into the neighbor-sum matmul lhsT: J = wt * I2/...,
    acc[v,:] += J.T @ gelu  (psum-resident across the whole kernel).
  * all matmuls bf16 (1 cyc/row warm, FWL weight loads).

Per block of 512 edge rows (8 w x 64 v), per core:
  xt [128,6,512] bf16 <- DMA slice of X^T
  gates^T = Wg[i|g|o].T @ xt          (6 chunks x 6 k x 512 c)
  h2 = (tanh(o/2)+1)*tanh(sig(i)*tanh(g))  (ACT tanh x4/half + DVE stt, bf16)
  pw = (Wl/2).T @ h2 ; wt_t = tanh(pw/2 + bl/2)  (DRAM-bounce reshape)
  m[p,f] = sum_k xt_k.T @ Wc_k + S_idx.T @ P2    (2 psum banks per row-tile)
  stats: bn_stats/bn_aggr; istd = newton_rsqrt(var+eps)
  gl = gelu(istd*m - mu*istd)  bf16
  acc += J.T @ gl,  J = (wt_t+1) * I2/2
out = node + acc.
"""
import numpy as np
import ml_dtypes
from contextlib import ExitStack

import concourse.mybir as mybir
import concourse.tile as tile
from concourse import bacc
from concourse.bass_utils import run_bass_kernel_spmd

F32 = mybir.dt.float32
BF16 = mybir.dt.bfloat16
FP8 = mybir.dt.float8e4
U32 = mybir.dt.uint32
I32 = mybir.dt.int32
AF = mybir.ActivationFunctionType
OP = mybir.AluOpType
DR = mybir.MatmulPerfMode.DoubleRow

B = 8           # batch == number of cores
N = 64          # nodes
D = 768         # feature dim
H = 256         # lstm hidden
ROWS = N * N    # 4096 edge rows per core
BLK = 512       # rows per block (8 w x 64 v)
NBLK = ROWS // BLK
TPB = BLK // 128  # row-tiles per block
KD = D // 128
LN_EPS = 1e-12
BFDT = ml_dtypes.bfloat16


def build(apply_lng=False, apply_lnb=False, reps=1, v=None):
    v = {**dict(xt_bufs=3, psg_bufs=3, psm_bufs=2, ms_bufs=10, gl_bufs=4,
                tmp_bufs=5, newton=2, fp8=True,
                skip_gates=False, skip_merge=False, skip_acc=False,
                dma2=False), **(v or {})}
    nc = bacc.Bacc(None)

    xte = nc.dram_tensor("xte", (D, ROWS), BF16, kind="ExternalInput")
    # fp8 copies for the DoubleRow gates matmuls: rows ordered
    # (k2, i, p) -> feature k2*256 + i*128 + p, matching wg8 packing
    x8e = nc.dram_tensor("x8e", (D, ROWS), FP8, kind="ExternalInput")
    wg8d = nc.dram_tensor("wg8", (128, KD // 2, 2, D), FP8, kind="ExternalInput")
    p2d = nc.dram_tensor("p2", (N, D), BF16, kind="ExternalInput")
    noded = nc.dram_tensor("node", (N, D), F32, kind="ExternalInput")
    wcd = nc.dram_tensor("wc", (D, D), BF16, kind="ExternalInput")
    wgpd = nc.dram_tensor("wgp", (D, D), BF16, kind="ExternalInput")
    bg6d = nc.dram_tensor("bg6", (128, 6), F32, kind="ExternalInput")
    wl2d = nc.dram_tensor("wl2", (H, 1), BF16, kind="ExternalInput")
    bl2d = nc.dram_tensor("bl2", (1,), F32, kind="ExternalInput")
    lngd = nc.dram_tensor("ln_g", (D,), F32, kind="ExternalInput")
    lnbd = nc.dram_tensor("ln_b", (D,), F32, kind="ExternalInput")
    outd = nc.dram_tensor("out", (N, D), F32, kind="ExternalOutput")

    # I2/2 stacked identity [128, 64]: row p -> col p%64, scaled 0.5 so that
    # J = (wt_t+1)*I2/2 = sigmoid(pw+bl)*I2.
    i2h_np = np.tile(np.eye(N, dtype=np.float32), (2, 1)) * 0.5
    i2h_dram = nc.inline_tensor(i2h_np.astype(np.float32), name="i2h")
    # selection stack: S[j, idx*128 + p] = 1 iff j == blk*8 + 2*t + p//64,
    # idx = blk*TPB + t.  lhsT of the P2-add matmul.
    s_np = np.zeros((N, NBLK * TPB, 128), np.float32)
    for idx in range(NBLK * TPB):
        blk, t = divmod(idx, TPB)
        w0 = blk * 8 + 2 * t
        s_np[w0, idx, 0:64] = 1.0
        s_np[w0 + 1, idx, 64:128] = 1.0
    sbig_dram = nc.inline_tensor(s_np.reshape(N, -1).astype(BFDT), name="sbig")

    with tile.TileContext(nc) as tc, ExitStack() as ctx:
        W = ctx.enter_context(tc.tile_pool(name="W", bufs=1))          # persistent
        xtp = ctx.enter_context(tc.tile_pool(name="xt", bufs=v["xt_bufs"]))
        hp = ctx.enter_context(tc.tile_pool(name="h", bufs=2))
        tmp = ctx.enter_context(tc.tile_pool(name="tmp", bufs=v["tmp_bufs"]))
        lnp = ctx.enter_context(tc.tile_pool(name="ln", bufs=v["ms_bufs"]))
        glp = ctx.enter_context(tc.tile_pool(name="gl", bufs=v["gl_bufs"]))
        sml = ctx.enter_context(tc.tile_pool(name="sml", bufs=6))
        drp = ctx.enter_context(tc.tile_pool(name="dr", bufs=2, space="DRAM"))

        psg = ctx.enter_context(tc.tile_pool(name="psg", bufs=v["psg_bufs"], space="PSUM"))
        psm = ctx.enter_context(tc.tile_pool(name="psm", bufs=v["psm_bufs"], space="PSUM"))
        psf = ctx.enter_context(tc.tile_pool(name="psf", bufs=1, space="PSUM"))

        # ---------------- persistent weights / constants ----------------
        # per-k tiles so block 0's first matmuls wait on per-k DMA granularity
        wg_sbs = []
        wc_sbs = []
        if v["fp8"]:
            wg8_sb = W.tile([128, KD // 2, 2, D], FP8, tag="wg8")
            nc.sync.dma_start(wg8_sb[:], wg8d[:])
        else:
            for k in range(KD):
                wgk = W.tile([128, D], BF16, tag=f"wg{k}", name=f"wg{k}")
                nc.sync.dma_start(wgk[:], wgpd[k * 128:(k + 1) * 128, :])
                wg_sbs.append(wgk)
        for k in range(KD):
            wck = W.tile([128, D], BF16, tag=f"wc{k}", name=f"wc{k}")
            nc.gpsimd.dma_start(wck[:], wcd[k * 128:(k + 1) * 128, :])
            wc_sbs.append(wck)

        sb_sb = W.tile([N, NBLK * TPB, 128], BF16, tag="sbig")
        nc.gpsimd.dma_start(sb_sb[:], sbig_dram[:].rearrange("j (i r) -> j i r", r=128))
        p2_sb = W.tile([N, D], BF16, tag="p2")
        nc.gpsimd.dma_start(p2_sb[:], p2d[:])

        wl_sb = W.tile([128, 2, 1], BF16, tag="wl")
        nc.sync.dma_start(wl_sb[:], wl2d[:].rearrange("(k p) a -> p k a", p=128))
        bg_sb = W.tile([128, 6], F32, tag="bg")
        nc.sync.dma_start(bg_sb[:], bg6d[:])
        bl2_sb = W.tile([128, 1], F32, tag="bl2")
        nc.sync.dma_start(bl2_sb[:], bl2d[:].partition_broadcast(128))
        i2h_sb = W.tile([128, N], F32, tag="i2h")
        nc.sync.dma_start(i2h_sb[:], i2h_dram[:])
        node_sb = W.tile([N, D], F32, tag="node")
        nc.sync.dma_start(node_sb[:], noded[:])
        if apply_lng:
            gfull = W.tile([128, D], F32, tag="gfull")
            nc.sync.dma_start(gfull[:], lngd[:].partition_broadcast(128))
        if apply_lnb:
            bfull = W.tile([128, D], F32, tag="bfull")
            nc.sync.dma_start(bfull[:], lnbd[:].partition_broadcast(128))

        # final accumulator, one bank: partitions 0:64 = lo half, 64:128 = hi
        acc_lo = acc_hi = None
        if not v["skip_acc"]:
            acc = psf.tile([128, 384], F32, tag="acc")
            acc_lo = acc[0:N, :]
            acc_hi = acc[N:128, :]

        out_sb = W.tile([N, D], F32, tag="out")
        if v["skip_merge"]:
            dmy_ms = W.tile([128, 2, 384], F32, tag="dmy")
            nc.gpsimd.memset(dmy_ms[:], 0.25)

        xview = xte[:].rearrange("(k p) c -> p k c", p=128)
        if v["fp8"]:
            x8view = x8e[:].rearrange("(k2 i p) c -> p k2 i c", p=128, i=2)

        # ---------------- main loop (body repeated `reps` times) ----------
        def body():
            # software-pipelined: block b's LN+gelu+neighbor-sum ("phase B") is
            # emitted interleaved into block b+1's gates matmuls so the
            # in-order PE queue never waits on the stats->rsqrt->gelu chain.
            def phase_b(st, t):
                if v["skip_acc"]:
                    return
                (mss, mvall, yb, nba, wt_t, blk) = st
                ms = mss[t]
                gl = glp.tile([128, 2, 384], BF16, tag="gl")
                if v["skip_merge"]:
                    nc.scalar.activation(gl[:], ms[:], AF.Gelu)
                elif not (apply_lng or apply_lnb):
                    nc.scalar.activation(gl[:], ms[:], AF.Gelu,
                                         scale=yb[:, t:t + 1], bias=nba[:, t:t + 1])
                else:
                    y = lnp.tile([128, 2, 384], F32, tag="y")
                    nc.vector.tensor_scalar(y[:], ms[:], mvall[:, t, 0:1],
                                            yb[:, t:t + 1], OP.subtract, OP.mult)
                    gview = lambda g: g[:].rearrange("p (h f) -> p h f", h=2)
                    if apply_lng:
                        z = lnp.tile([128, 2, 384], F32, tag="y")
                        nc.vector.tensor_tensor(z[:], y[:], gview(gfull), OP.mult)
                        y = z
                    if apply_lnb:
                        z = lnp.tile([128, 2, 384], F32, tag="y")
                        nc.vector.tensor_tensor(z[:], y[:], gview(bfull), OP.add)
                        y = z
                    nc.scalar.activation(gl[:], y[:], AF.Gelu)

                J = sml.tile([128, N], BF16, tag="J")
                nc.vector.scalar_tensor_tensor(
                    J[:], wt_t[:, t:t + 1].broadcast_to((128, N)), 1.0,
                    i2h_sb[:], OP.add, OP.mult)
                first = blk == 0 and t == 0
                last = blk == NBLK - 1 and t == TPB - 1
                nc.tensor.matmul(acc_lo, J[:], gl[:, 0, :],
                                 start=first, stop=last, skip_group_check=True)
                nc.tensor.matmul(acc_hi, J[:], gl[:, 1, :],
                                 start=first, stop=last, skip_group_check=True)

            prev = None
            for blk in range(NBLK):
                xt = xtp.tile([128, KD, BLK], BF16, tag="xt")
                if v["fp8"]:
                    xt8 = xtp.tile([128, KD // 2, 2, BLK], FP8, tag="xt8")
                if True:
                    nc.sync.dma_start(xt[:, 0:3, :],
                                      xview[:, 0:3, blk * BLK:(blk + 1) * BLK])
                    nc.gpsimd.dma_start(xt[:, 3:6, :],
                                        xview[:, 3:6, blk * BLK:(blk + 1) * BLK])
                    if v["fp8"]:
                        nc.sync.dma_start(xt8[:],
                                          x8view[:, :, :, blk * BLK:(blk + 1) * BLK])
                    if v["dma2"]:
                        xd = xtp.tile([128, KD, BLK], BF16, tag="xd")
                        nc.sync.dma_start(xd[:, 0:3, :],
                                          xview[:, 0:3, blk * BLK:(blk + 1) * BLK])
                        nc.gpsimd.dma_start(xd[:, 3:6, :],
                                            xview[:, 3:6, blk * BLK:(blk + 1) * BLK])

                # ---- gates (all-tanh rewrite; sig(x)=0.5*tanh(x/2)+0.5) ----
                # chunk order in wgp/bg6: i0 i1 g0 g1 o0 o1 (i,o biases halved)
                def gate_mm(cj):
                    pg = psg.tile([128, BLK], F32, tag="s1")
                    if v["fp8"]:
                        for k2 in range(KD // 2):
                            nc.tensor.matmul(pg[:],
                                             wg8_sb[:, k2, :, cj * 128:(cj + 1) * 128],
                                             xt8[:, k2, :, :], start=(k2 == 0),
                                             stop=(k2 == KD // 2 - 1), perf_mode=DR)
                    else:
                        for k in range(KD):
                            nc.tensor.matmul(pg[:], wg_sbs[k][:, cj * 128:(cj + 1) * 128],
                                             xt[:, k, :], start=(k == 0), stop=(k == KD - 1))
                    return pg

                if v["skip_gates"]:
                    wt_t = sml.tile([128, TPB], F32, tag="wtt")
                    nc.gpsimd.memset(wt_t[:], 0.0)
                    if prev is not None:
                        for tt in range(TPB):
                            phase_b(prev, tt)
                h_sb = hp.tile([128, 2, BLK], BF16, tag="h")
                for half in range(2 if not v["skip_gates"] else 0):
                    pg_i = gate_mm(half)
                    tan_i = tmp.tile([128, BLK], BF16, tag="tmp")
                    nc.scalar.activation(tan_i[:], pg_i[:], AF.Tanh, scale=0.5,
                                         bias=bg_sb[:, half:half + 1])
                    pg_g = gate_mm(2 + half)
                    tan_g = tmp.tile([128, BLK], BF16, tag="tmp")
                    nc.scalar.activation(tan_g[:], pg_g[:], AF.Tanh,
                                         bias=bg_sb[:, 2 + half:3 + half])
                    c_t = tmp.tile([128, BLK], BF16, tag="tmp")
                    nc.vector.scalar_tensor_tensor(c_t[:], tan_i[:], 1.0, tan_g[:],
                                                   OP.add, OP.mult)
                    tan_c = tmp.tile([128, BLK], BF16, tag="tmp")
                    nc.scalar.activation(tan_c[:], c_t[:], AF.Tanh, scale=0.5)
                    pg_o = gate_mm(4 + half)
                    tan_o = tmp.tile([128, BLK], BF16, tag="tmp")
                    nc.scalar.activation(tan_o[:], pg_o[:], AF.Tanh, scale=0.5,
                                         bias=bg_sb[:, 4 + half:5 + half])
                    nc.vector.scalar_tensor_tensor(h_sb[:, half, :], tan_o[:], 1.0,
                                                   tan_c[:], OP.add, OP.mult)
                    # interleave prev block's gelu+acc with this block's gates
                    if prev is not None:
                        phase_b(prev, 2 * half)
                        phase_b(prev, 2 * half + 1)

                # ---- merge: m = X @ Wc + P2[w]; stats per row-tile ----
                mss = []
                mvall = (None if v["skip_merge"] else
                         sml.tile([128, TPB, 2], F32, tag="mv"))
                for t in range(TPB):
                    if v["skip_merge"]:
                        mss.append(dmy_ms)
                        continue
                    idx = blk * TPB + t
                    pm = psm.tile([128, 2, BLK], F32, tag="pm")
                    for hf in range(2):
                        o = pm[:, hf, 0:384]
                        nc.tensor.matmul(o, sb_sb[:, idx, :],
                                         p2_sb[:, hf * 384:(hf + 1) * 384],
                                         start=True, stop=False)
                        for k in range(KD):
                            nc.tensor.matmul(o, xt[:, k, t * 128:(t + 1) * 128],
                                             wc_sbs[k][:, hf * 384:(hf + 1) * 384],
                                             start=False, stop=(k == KD - 1))
                    ms = lnp.tile([128, 2, 384], F32, tag="ms", name=f"ms_{blk}_{t}")
                    nc.scalar.activation(ms[:], pm[:, :, 0:384], AF.Identity)
                    stats = sml.tile([128, 2, 6], F32, tag="st")
                    nc.vector.bn_stats(stats[:, 0, :], ms[:, 0, :])
                    nc.vector.bn_stats(stats[:, 1, :], ms[:, 1, :])
                    nc.vector.bn_aggr(mvall[:, t, :], stats[:])
                    mss.append(ms)

                # ---- edge weight: pw = (Wl/2).T @ h2 -> [1, 512] ----
                # (emitted after merge so the PE never waits on the tanh chain)
                if v["skip_gates"]:
                    pgw = None
                else:
                    pgw = psg.tile([128, BLK], F32, tag="s1")
                if not v["skip_gates"]:
                    pw = pgw[0:1, :]
                    for k in range(2):
                        nc.tensor.matmul(pw, wl_sb[:, k, :], h_sb[:, k, :],
                                         start=(k == 0), stop=(k == 1))
                    wrow = sml.tile([1, BLK], F32, tag="wrow")
                    nc.vector.tensor_copy(wrow[:], pw)
                    wdr = drp.tile([1, BLK], F32, tag="wdr")
                    nc.gpsimd.dma_start(wdr[:], wrow[:])
                    wtp = sml.tile([128, TPB], F32, tag="wtp")
                    nc.gpsimd.dma_start(wtp[:],
                                        wdr[0:1, :].rearrange("a (t p) -> (a p) t", p=128))
                    wt_t = sml.tile([128, TPB], F32, tag="wtt")
                    nc.scalar.activation(wt_t[:], wtp[:], AF.Tanh, scale=0.5,
                                         bias=bl2_sb[:])

                # ---- istd = rsqrt(var + eps): bit-hack seed + newton ----
                yb = nba = None
                if not v["skip_merge"]:
                    veps = sml.tile([128, TPB], F32, tag="veps")
                    yb = sml.tile([128, TPB], F32, tag="yb")
                    nba = sml.tile([128, TPB], F32, tag="nba")
                    nc.vector.tensor_scalar(veps[:], mvall[:, :, 1], LN_EPS, None, OP.add)
                    # y0 = bits(0x5f3759df - (bits(v) >> 1)) = magic+1 + ~(v>>1)
                    nc.vector.tensor_scalar(yb[:].bitcast(U32), veps[:].bitcast(U32),
                                            1, 0xFFFFFFFF,
                                            OP.logical_shift_right, OP.bitwise_xor)
                    # int32 add: uint32 ALU add saturates on device, int32 is
                    # in-range here (~(v>>1) is negative, result positive)
                    nc.vector.tensor_scalar(yb[:].bitcast(I32), yb[:].bitcast(I32),
                                            0x5f3759e0, None, OP.add)
                    for _ in range(v["newton"]):
                        a = sml.tile([128, TPB], F32, tag="nta")
                        nc.vector.tensor_tensor(a[:], veps[:], yb[:], OP.mult)
                        nc.vector.tensor_tensor(a[:], a[:], yb[:], OP.mult)
                        nc.vector.tensor_scalar(a[:], a[:], -0.5, 1.5, OP.mult, OP.add)
                        nc.vector.tensor_tensor(yb[:], yb[:], a[:], OP.mult)
                    nc.vector.scalar_tensor_tensor(nba[:], mvall[:, :, 0], -1.0,
                                                   yb[:], OP.mult, OP.mult)

                prev = (mss, mvall, yb, nba, wt_t, blk)

            for t in range(TPB):
                phase_b(prev, t)

            # ---- residual + store ----
            if v["skip_acc"]:
                nc.vector.tensor_copy(out_sb[:], node_sb[:])
            else:
                nc.vector.scalar_tensor_tensor(out_sb[:, 0:384], acc_lo, 0.0,
                                               node_sb[:, 0:384], OP.add, OP.add)
                nc.vector.scalar_tensor_tensor(out_sb[:, 384:768], acc_hi, 0.0,
                                               node_sb[:, 384:768], OP.add, OP.add)
            nc.sync.dma_start(outd[:], out_sb[:])

        if reps == 1:
            body()
        else:
            with tc.For_i(0, reps, 1):
                body()

    nc.finalize()
    return nc


_CACHE = {}
VOPT = None


def _get_nc(flags, reps=1):
    key = (flags, reps, repr(VOPT))
    if key not in _CACHE:
        _CACHE[key] = build(apply_lng=flags[0], apply_lnb=flags[1],
                            reps=reps, v=VOPT)
    return _CACHE[key]


def _flags(inputs):
    return (not bool(np.allclose(inputs["ln_g"], 1.0)),
            bool(np.any(inputs["ln_b"])))


def _in_maps(inputs):
    e = np.asarray(inputs["edge_features"], np.float32)       # [8,64,64,768]
    nf = np.ascontiguousarray(inputs["node_features"], np.float32)
    Wg = np.asarray(inputs["W_gates"], np.float32)            # [768,1024]
    bgv = np.asarray(inputs["b_gates"], np.float32)
    Wl = np.asarray(inputs["W_lout"], np.float32)             # [256,1]
    blv = np.asarray(inputs["b_lout"], np.float32)
    Wm = np.asarray(inputs["W_msg"], np.float32)              # [1536,768]
    bmv = np.asarray(inputs["b_msg"], np.float32)
    Wr = np.asarray(inputs["W_mrg"], np.float32)              # [768,768]
    brv = np.asarray(inputs["b_mrg"], np.float32)

    # X^T per core: [768, 4096] bf16, column = w*64 + v
    xTf = np.ascontiguousarray(e.transpose(0, 3, 2, 1)).reshape(B, D, ROWS)
    xT = xTf.astype(BFDT)
    x8 = xTf.astype(ml_dtypes.float8_e4m3)
    # P2[w] = (node_w @ Wm_top + b_msg) @ W_mrg + b_mrg  (per core, host)
    p2 = (((nf @ Wm[:D] + bmv) @ Wr) + brv).astype(BFDT)      # [8,64,768]
    # fused message-bottom x merge weight
    wc = (Wm[D:].astype(np.float64) @ Wr.astype(np.float64)).astype(np.float32).astype(BFDT)
    # gates packed [i|g|o] (f-gate dropped: c0 = 0)
    wgpf = np.ascontiguousarray(
        np.concatenate([Wg[:, 0:256], Wg[:, 512:1024]], axis=1))
    wgp = wgpf.astype(BFDT)
    # DoubleRow packing: wg8[p, k2, i, m] = wgp[k2*256 + i*128 + p, m]
    wg8 = np.ascontiguousarray(
        wgpf.reshape(3, 2, 128, D).transpose(2, 0, 1, 3)).astype(ml_dtypes.float8_e4m3)
    bg6 = np.stack([bgv[0:128] * .5, bgv[128:256] * .5, bgv[512:640],
                    bgv[640:768], bgv[768:896] * .5, bgv[896:1024] * .5],
                   axis=1).astype(np.float32)                 # [128, 6]
    wl2 = (Wl * 0.5).astype(BFDT)
    bl2 = (blv * 0.5).astype(np.float32)
    lg = np.ascontiguousarray(inputs["ln_g"], np.float32)
    lb = np.ascontiguousarray(inputs["ln_b"], np.float32)

    shared = dict(wc=wc, wgp=wgp, wg8=wg8, bg6=bg6, wl2=wl2, bl2=bl2,
                  ln_g=lg, ln_b=lb)
    return [dict(xte=xT[b], x8e=x8[b], p2=p2[b], node=nf[b], **shared)
            for b in range(B)]


def kernel(**inputs):
    nc = _get_nc(_flags(inputs))
    res = run_bass_kernel_spmd(nc, _in_maps(inputs), list(range(B)))
    return np.stack([res.results[b]["out"] for b in range(B)]).astype(np.float32)


def run_timed(inputs, reps):
    """Run the reps-looped variant once; returns (output, wall_seconds)."""
    import time
    nc = _get_nc(_flags(inputs), reps=reps)
    maps = _in_maps(inputs)
    t0 = time.time()
    res = run_bass_kernel_spmd(nc, maps, list(range(B)))
    dt = time.time() - t0
    out = np.stack([res.results[b]["out"] for b in range(B)]).astype(np.float32)
    return out, dt
